# revision 4
# baseline (speedup 1.0000x reference)
"""Bidirectional GQA self-attention (B=4, T=2048, C=2048, 16 q-heads /
4 kv-heads, RoPE) on 8 Trainium2 NeuronCores — v2.

Sharding: row-data-parallel over (batch, token-half): core c handles batch
c//2, query tokens [(c%2)*1024, (c%2)*1024+1024).

v2 vs baseline:
- fp16 on-chip everywhere (PE rate unchanged, DVE 2x, DMA/SBUF halved).
- Few large DMAs (SP issue-rate was the phase-A bottleneck) and no
  redundant weight reloads.
- Merged pipeline: q-projection of head h+1 is interleaved into the
  attention kc-loop of head h so PE never idles on exp latency.
- Softmax denominator: DVE add-chain over p tiles + gpsimd
  partition_all_reduce; no ones-matmuls, no broadcast.
- mode="coll": A1 computes K/V for own half, pairwise AllGather, re-import
  both halves in rank order.  mode="zero": full-batch K/V locally (timing
  twin; ~54us more PE work, no collective).
"""
import sys

sys.path.insert(0, "/opt/trn_rl_repo")

import numpy as np

import concourse.bass as bass
import concourse.bass_isa as bass_isa
import concourse.mybir as mybir
import concourse.tile as tile
from concourse import bacc
from concourse.bass_utils import run_bass_kernel_spmd

B, T, C = 4, 2048, 2048
NH, NKV, HD = 16, 4, 128
REP = NH // NKV
TQ = 1024
NCORES = 8
SCALE = 1.0 / np.sqrt(HD)

F16 = mybir.dt.float16
F32 = mybir.dt.float32
MULT = mybir.AluOpType.mult
ADD = mybir.AluOpType.add
EXP = mybir.ActivationFunctionType.Exp
RADD = bass_isa.ReduceOp.add

SWAP16 = [(i + 16) % 32 for i in range(32)]

NCK = C // 128        # 16 contraction chunks over C
NTK = T // 128        # 16 key chunks of 128 tokens


def _build(mode="coll", repeat=1):
    nc = bacc.Bacc("TRN2", target_bir_lowering=False, debug=False)

    xq = nc.dram_tensor("xq", [C, TQ], F16, kind="ExternalInput")
    wq = nc.dram_tensor("wq", [C, NH * HD], F16, kind="ExternalInput")
    wkv = nc.dram_tensor("wkv", [C, 2 * NKV * HD], F16, kind="ExternalInput")
    wp = nc.dram_tensor("wp", [C, C], F16, kind="ExternalInput")
    csq = nc.dram_tensor("csq", [128, TQ], F16, kind="ExternalInput")
    ssq = nc.dram_tensor("ssq", [128, TQ], F16, kind="ExternalInput")
    TK = TQ if mode == "coll" else T
    csk = nc.dram_tensor("csk", [128, TK], F16, kind="ExternalInput")
    ssk = nc.dram_tensor("ssk", [128, TK], F16, kind="ExternalInput")
    if mode == "zero":
        xt = nc.dram_tensor("xt", [C, T], F16, kind="ExternalInput")
    else:
        kv_own = nc.dram_tensor("kv_own", [16, 128, 512], F16)
        kv_gath = nc.dram_tensor("kv_gath", [32, 128, 512], F16)
    out = nc.dram_tensor("out", [TQ, C], F32, kind="ExternalOutput")

    import contextlib

    with tile.TileContext(nc) as tc:
        rep_cm = tc.For_i(0, repeat, 1) if repeat > 1 else contextlib.nullcontext()
        with (
            rep_cm,
            tc.tile_pool(name="persist", bufs=1) as pp,
            tc.tile_pool(name="yt", bufs=16) as ytp,
            tc.tile_pool(name="wqp", bufs=(3 if mode == "coll" else 2)) as wqp,
        ):
            wqg_tiles = {}
            wph_tiles = {}

            def load_wqg(g, eng=None):
                eng = eng or nc.sync
                t = wqp.tile([128, NCK * 256], F16, tag="wqg",
                             name=f"wqg{g}")
                eng.dma_start(
                    t[:].rearrange("p (ck f) -> p ck f", ck=NCK),
                    wq[:, g * 256:(g + 1) * 256].rearrange(
                        "(ck p) f -> p ck f", p=128))
                wqg_tiles[g] = t
            # ---- persistent tiles ----
            xall_tiles = [pp.tile([128, 4 * TQ], F16, tag=f"xall{j}",
                                  name=f"xall{j}") for j in range(4)]

            def xsl(ck, lo, ln):
                return xall_tiles[ck // 4][:, (ck % 4) * TQ + lo:
                                           (ck % 4) * TQ + lo + ln]

            def load_xall():
                for j in range(4):
                    nc.sync.dma_start(
                        xall_tiles[j][:].rearrange(
                            "p (ck f) -> p ck f", ck=4),
                        xq[j * 512:(j + 1) * 512, :].rearrange(
                            "(ck p) f -> p ck f", p=128))

            if mode == "coll":
                load_xall()
            vfull = pp.tile([128, NTK * 512], F16, tag="vfull")
            k_tiles = [pp.tile([128, T], F16, tag=f"kt{m}", name=f"kT{m}")
                       for m in range(NKV)]
            csq_t = pp.tile([128, TQ], F16, tag="csq")
            ssq_t = pp.tile([128, TQ], F16, tag="ssq")

            # ---------------- phase A1: K/V ----------------
            with (
                tc.tile_pool(name="wkvp", bufs=1) as wkvp,
                tc.tile_pool(name="xtp", bufs=2) as xtp,
                tc.tile_pool(name="cskp", bufs=1) as cskp,
                tc.tile_pool(name="rpk", bufs=3) as rpk,
                tc.tile_pool(name="psA", bufs=1, space="PSUM") as psA,
            ):
                wkv_tiles = [wkvp.tile([128, 2 * 1024], F16,
                                       tag=f"wkv{j}", name=f"wkv{j}")
                             for j in range(8)]

                def wkvsl(ck, lo, ln):
                    return wkv_tiles[ck // 2][:, (ck % 2) * 1024 + lo:
                                              (ck % 2) * 1024 + lo + ln]

                xtq_tiles = {}

                def load_xtq(qtr, eng):
                    subs = []
                    for j in range(4):
                        t = xtp.tile([128, 4 * 512], F16, tag=f"xtq{j}",
                                     name=f"xtq{qtr}_{j}")
                        eng.dma_start(
                            t[:].rearrange("p (ck f) -> p ck f", ck=4),
                            xt[j * 512:(j + 1) * 512,
                               qtr * 512:(qtr + 1) * 512].rearrange(
                                "(ck p) f -> p ck f", p=128))
                        subs.append(t)
                    xtq_tiles[qtr] = subs

                if mode == "zero":
                    load_xtq(0, nc.sync)
                for j in range(8):
                    nc.gpsimd.dma_start(
                        wkv_tiles[j][:].rearrange(
                            "p (ck f) -> p ck f", ck=2),
                        wkv[j * 256:(j + 1) * 256, :].rearrange(
                            "(ck p) f -> p ck f", p=128))
                csk_t = cskp.tile([128, TK], F16, tag="csk")
                ssk_t = cskp.tile([128, TK], F16, tag="ssk")
                nc.sync.dma_start(csk_t[:], csk[:])
                nc.sync.dma_start(ssk_t[:], ssk[:])
                nc.sync.dma_start(csq_t[:], csq[:])
                nc.sync.dma_start(ssq_t[:], ssq[:])
                load_wqg(0, nc.scalar)
                load_wqg(1, nc.scalar)
                if mode == "zero":
                    load_xtq(1, nc.scalar)
                    load_xtq(2, nc.scalar)
                    load_xtq(3, nc.gpsimd)
                    load_xall()

                nq = 2 if mode == "coll" else 4
                for qtr in range(nq):
                    if mode == "zero":
                        xtq = xtq_tiles[qtr]
                    k_ps = [psA.tile([128, 512], F32, tag=f"kps{i}", name=f"kps{i}")
                            for i in range(NKV)]
                    v_ps = [psA.tile([128, 512], F32, tag=f"vps{i}", name=f"vps{i}")
                            for i in range(4)]
                    for ck in range(NCK):
                        if mode == "coll":
                            xa = xsl(ck, qtr * 512, 512)
                        else:
                            xa = xtq[ck // 4][:, (ck % 4) * 512:
                                              (ck % 4 + 1) * 512]
                        st = (ck == 0)
                        sp = (ck == NCK - 1)
                        for tv in range(4):
                            nc.tensor.matmul(
                                v_ps[tv][:], xa[:, tv * 128:(tv + 1) * 128],
                                wkvsl(ck, 512, 512),
                                start=st, stop=sp)
                        for m in range(NKV):
                            nc.tensor.matmul(
                                k_ps[m][:],
                                wkvsl(ck, m * 128, 128),
                                xa, start=st, stop=sp)
                    # v: psum -> vfull (fp16) at local chunk position
                    for tv in range(4):
                        kcg = qtr * 4 + tv
                        nc.scalar.copy(
                            vfull[:, kcg * 512:(kcg + 1) * 512], v_ps[tv][:])
                    # k: rope from psum into k_tiles at local token offset
                    for m in range(NKV):
                        tok0 = qtr * 512
                        tmpk = rpk.tile([128, 512], F16, tag="tmpk")
                        nc.scalar.copy(tmpk[:], k_ps[m][:])
                        bb = rpk.tile([128, 512], F16, tag="rkb")
                        cc = rpk.tile([128, 512], F16, tag="rkc")
                        dst = k_tiles[m][:, tok0:tok0 + 512]
                        nc.vector.tensor_tensor(
                            dst, tmpk[:], csk_t[:, tok0:tok0 + 512], MULT)
                        nc.vector.tensor_tensor(
                            bb[:], tmpk[:], ssk_t[:, tok0:tok0 + 512], MULT)
                        nc.vector.stream_shuffle(cc[:], bb[:], SWAP16)
                        nc.vector.tensor_tensor(dst, dst, cc[:], ADD)

            if mode == "coll":
                # export own half (local chunks 0..7): k rows 0..7, v rows
                # 8..15 of kv_own
                for m in range(NKV):
                    nc.sync.dma_start(
                        kv_own[2 * m:2 * m + 2, :, :].rearrange(
                            "s p f -> p s f"),
                        k_tiles[m][:, 0:TQ].rearrange("p (s f) -> p s f", s=2))
                nc.sync.dma_start(
                    kv_own[8:16, :, :].rearrange("s p f -> p s f"),
                    vfull[:, 0:8 * 512].rearrange("p (s f) -> p s f", s=8))
                nc.gpsimd.collective_compute(
                    "AllGather", mybir.AluOpType.bypass,
                    replica_groups=[[0, 1], [2, 3], [4, 5], [6, 7]],
                    ins=[kv_own[:]], outs=[kv_gath[:]])
                # re-import BOTH halves in pair-rank order (= global token
                # order, matching the per-core csq/csk slicing).
                for m in range(NKV):
                    for hh in range(2):
                        nc.sync.dma_start(
                            k_tiles[m][:, hh * TQ:(hh + 1) * TQ].rearrange(
                                "p (s f) -> p s f", s=2),
                            kv_gath[hh * 16 + 2 * m:hh * 16 + 2 * m + 2,
                                    :, :].rearrange("s p f -> p s f"))
                for hh in range(2):
                    nc.sync.dma_start(
                        vfull[:, hh * 4096:(hh + 1) * 4096].rearrange(
                            "p (s f) -> p s f", s=8),
                        kv_gath[hh * 16 + 8:hh * 16 + 16, :, :].rearrange(
                            "s p f -> p s f"))

            if mode == "coll":
                load_wqg(2)
                load_wqg(3)

            # ---------------- merged phase: per-head qproj + attention ----
            wpp_cm = tc.tile_pool(name="wph", bufs=2)
            wpp = wpp_cm.__enter__()
            with (
                tc.tile_pool(name="qt", bufs=(6 if mode == "coll" else 3)) as qtp,
                tc.tile_pool(name="pt", bufs=4) as ptp,
                tc.tile_pool(name="rpq", bufs=1) as rpq,
                tc.tile_pool(name="acc", bufs=1) as accp,
                tc.tile_pool(name="den", bufs=1) as denp,
                tc.tile_pool(name="psQ", bufs=1, space="PSUM") as psQ,
                tc.tile_pool(name="psS", bufs=2, space="PSUM") as psS,
                tc.tile_pool(name="psY", bufs=2, space="PSUM") as psY,
            ):
                q_ps_of = {}

                def emit_qp(h, lo, hi):
                    """Emit qproj matmuls [lo,hi) of head h (32 total)."""
                    if lo == 0:
                        q_ps_of[h] = psQ.tile([128, TQ], F32, tag="qps",
                                              name=f"qps{h}")
                    q_ps = q_ps_of[h]
                    wqg = wqg_tiles[h // 2]
                    hsl = (h % 2) * 128
                    for i in range(lo, hi):
                        ck, t = divmod(i, 2)
                        nc.tensor.matmul(
                            q_ps[:, t * 512:(t + 1) * 512],
                            wqg[:, ck * 256 + hsl:ck * 256 + hsl + 128],
                            xsl(ck, t * 512, 512),
                            start=(ck == 0), stop=(ck == NCK - 1))

                q_t_of = {}

                def emit_rope_q(h):
                    q_ps = q_ps_of.pop(h)
                    tmpq = rpq.tile([128, TQ], F16, tag="tmpq")
                    nc.scalar.copy(tmpq[:], q_ps[:])
                    bb = rpq.tile([128, TQ], F16, tag="rqb")
                    cc = rpq.tile([128, TQ], F16, tag="rqc")
                    q_t = qtp.tile([128, TQ], F16, tag="qt")
                    nc.vector.tensor_tensor(q_t[:], tmpq[:], csq_t[:], MULT)
                    nc.vector.tensor_tensor(bb[:], tmpq[:], ssq_t[:], MULT)
                    nc.vector.stream_shuffle(cc[:], bb[:], SWAP16)
                    nc.vector.tensor_tensor(q_t[:], q_t[:], cc[:], ADD)
                    q_t_of[h] = q_t

                y_tiles = []

                def emit_sc(h, kc, s_list, p_list, pair_list):
                    g = h // 4
                    q_t = q_t_of[h]
                    s_ps = psS.tile([128, 1024], F32, tag="sps")
                    for t in range(2):
                        nc.tensor.matmul(
                            s_ps[:, t * 512:(t + 1) * 512],
                            k_tiles[g][:, kc * 128:(kc + 1) * 128],
                            q_t[:, t * 512:(t + 1) * 512],
                            start=True, stop=True)
                    if kc % 2 == 0:
                        pair = ptp.tile([128, 2048], F16, tag="pt")
                        pair_list.append(pair)
                    else:
                        pair = pair_list[kc // 2]
                    half = (kc % 2) * 1024
                    nc.scalar.activation(pair[:, half:half + 1024],
                                         s_ps[:], EXP)
                    s_list.append(s_ps)
                    p_list.append(pair[:, half:half + 1024])

                # lead-in: full qproj + rope for heads 0..3 (hides the
                # KV AllGather + import latency in coll mode)
                LEAD = 4 if mode == "coll" else 1
                for hh in range(LEAD):
                    emit_qp(hh, 0, 32)
                    emit_rope_q(hh)

                for h in range(NH):
                    g = h // 4
                    if h % 2 == 0 and h // 2 + (2 if mode == "zero" else LEAD) < 8:
                        load_wqg(h // 2 + (2 if mode == "zero" else LEAD))
                    s_list, p_list, pair_list = [], [], []
                    y_ps = [psY.tile([128, 512], F32, tag="yps", name=f"yps{h}_{i}")
                            for i in range(2)]
                    acc2 = accp.tile([128, 2048], F16, tag="acc")
                    emit_sc(h, 0, s_list, p_list, pair_list)
                    emit_sc(h, 1, s_list, p_list, pair_list)
                    for kc in range(NTK):
                        if h + LEAD < NH:
                            emit_qp(h + LEAD, kc * 2, kc * 2 + 2)
                        # AV for kc
                        p_t = p_list[kc]
                        for t in range(2):
                            nc.tensor.matmul(
                                y_ps[t][:],
                                vfull[:, kc * 512 + g * 128:
                                      kc * 512 + (g + 1) * 128],
                                p_t[:, t * 512:(t + 1) * 512],
                                start=(kc == 0), stop=(kc == NTK - 1))
                        if kc + 2 < NTK:
                            emit_sc(h, kc + 2, s_list, p_list, pair_list)
                        # denominator: wide pair-adds on DVE (full pair
                        # tiles, so one add covers two key chunks)
                        if kc == 3:
                            nc.vector.tensor_tensor(
                                acc2[:], pair_list[0][:], pair_list[1][:],
                                ADD)
                        elif kc >= 4 and kc % 2 == 1:
                            nc.vector.tensor_tensor(
                                acc2[:], acc2[:], pair_list[kc // 2][:], ADD)
                    acc = accp.tile([128, 1024], F16, tag="accf")
                    nc.vector.tensor_tensor(
                        acc[:], acc2[:, 0:1024], acc2[:, 1024:2048], ADD)
                    dred = denp.tile([128, 1024], F32, tag="dred")
                    nc.gpsimd.partition_all_reduce(dred[:], acc[:], 128, RADD)
                    if h + LEAD < NH:
                        emit_rope_q(h + LEAD)
                    rcp = denp.tile([128, 1024], F32, tag="rcp")
                    nc.vector.reciprocal(rcp[:], dred[:])
                    y_t = ytp.tile([128, TQ], F16, tag="yt", name=f"yT{h}")
                    y_tiles.append(y_t)
                    for t in range(2):
                        nc.vector.tensor_tensor(
                            y_t[:, t * 512:(t + 1) * 512], y_ps[t][:],
                            rcp[:, t * 512:(t + 1) * 512], MULT)
                    if h in (10, 13):
                        # prefetch w_proj quarters for phase C
                        qn = 0 if h == 10 else 1
                        t = wpp.tile([128, NCK * 512], F16, tag="wph",
                                     name=f"wph{qn}")
                        nc.sync.dma_start(
                            t[:].rearrange("p (ck f) -> p ck f", ck=NCK),
                            wp[:, qn * 512:(qn + 1) * 512].rearrange(
                                "(ck p) f -> p ck f", p=128))
                        wph_tiles[qn] = t

            # ---------------- phase C: projection ----------------
            with (
                tc.tile_pool(name="oc", bufs=3) as ocp,
                tc.tile_pool(name="psC", bufs=3, space="PSUM") as psC,
            ):
                def load_wph(qn):
                    t = wpp.tile([128, NCK * 512], F16, tag="wph",
                                 name=f"wph{qn}")
                    nc.sync.dma_start(
                        t[:].rearrange("p (ck f) -> p ck f", ck=NCK),
                        wp[:, qn * 512:(qn + 1) * 512].rearrange(
                            "(ck p) f -> p ck f", p=128))
                    wph_tiles[qn] = t

                for qn in range(4):
                    if qn + 2 < 4:
                        load_wph(qn + 2)
                    wph = wph_tiles[qn]
                    for mt in range(TQ // 128):
                        o_ps = psC.tile([128, 512], F32, tag="ops")
                        for ck in range(NCK):
                            nc.tensor.matmul(
                                o_ps[:],
                                y_tiles[ck][:, mt * 128:(mt + 1) * 128],
                                wph[:, ck * 512:(ck + 1) * 512],
                                start=(ck == 0), stop=(ck == NCK - 1))
                        o_t = ocp.tile([128, 512], F32, tag="oc")
                        nc.scalar.copy(o_t[:], o_ps[:])
                        nc.sync.dma_start(
                            out[mt * 128:(mt + 1) * 128,
                                qn * 512:(qn + 1) * 512], o_t[:])
            wpp_cm.__exit__(None, None, None)
    return nc


_NC_CACHE = {}


def _get_nc(mode="coll", repeat=1):
    key = (mode, repeat)
    if key not in _NC_CACHE:
        nc = _build(mode, repeat)
        nc.compile()
        _NC_CACHE[key] = nc
    return _NC_CACHE[key]


def _head_perm():
    """col permutation within one head: new[qd*32 + e*16 + s] = old[2*(qd*16+s)+e]"""
    idx = np.empty(128, np.int64)
    for qd in range(4):
        for e in range(2):
            for s in range(16):
                idx[qd * 32 + e * 16 + s] = 2 * (qd * 16 + s) + e
    return idx


def make_in_maps(x, freqs_cis, w_qkv, w_proj, mode="coll"):
    x = np.asarray(x, dtype=np.float32)
    freqs_cis = np.asarray(freqs_cis, dtype=np.float32)
    w_qkv = np.asarray(w_qkv, dtype=np.float32)
    w_proj = np.asarray(w_proj, dtype=np.float32)

    hp = _head_perm()
    qperm = np.concatenate([h * 128 + hp for h in range(NH)])
    kperm = np.concatenate([h * 128 + hp for h in range(NKV)])
    wq = np.ascontiguousarray(
        w_qkv[:, :NH * HD][:, qperm].astype(np.float16))
    wk = w_qkv[:, NH * HD:NH * HD + NKV * HD][:, kperm]
    wv = w_qkv[:, NH * HD + NKV * HD:]
    wkv = np.ascontiguousarray(
        np.concatenate([wk, wv], axis=1).astype(np.float16))
    wp = np.ascontiguousarray(w_proj.astype(np.float16))

    cos = np.ascontiguousarray(freqs_cis[:, :, 0].T)  # [64, T]
    sin = np.ascontiguousarray(freqs_cis[:, :, 1].T)
    pair = np.empty(128, np.int64)
    sgn = np.empty(128, np.float32)
    for qd in range(4):
        for e in range(2):
            for s in range(16):
                row = qd * 32 + e * 16 + s
                pair[row] = qd * 16 + s
                sgn[row] = 1.0 if e == 0 else -1.0
    CS = np.ascontiguousarray(cos[pair])                 # [128, T]
    SS = np.ascontiguousarray(sin[pair] * sgn[:, None])  # [128, T]

    xT = [np.ascontiguousarray(x[b].T.astype(np.float16)) for b in range(B)]
    CSq = (CS * np.float32(SCALE)).astype(np.float16)
    SSq = (SS * np.float32(SCALE)).astype(np.float16)
    CSk = CS.astype(np.float16)
    SSk = SS.astype(np.float16)

    in_maps = []
    for c in range(NCORES):
        b, h = divmod(c, 2)
        sl = slice(h * TQ, (h + 1) * TQ)
        im = {
            "xq": np.ascontiguousarray(xT[b][:, sl]),
            "wq": wq, "wkv": wkv, "wp": wp,
            "csq": np.ascontiguousarray(CSq[:, sl]),
            "ssq": np.ascontiguousarray(SSq[:, sl]),
        }
        if mode == "coll":
            im["csk"] = np.ascontiguousarray(CSk[:, sl])
            im["ssk"] = np.ascontiguousarray(SSk[:, sl])
        else:
            im["csk"] = CSk
            im["ssk"] = SSk
            im["xt"] = xT[b]
        in_maps.append(im)
    return in_maps


def kernel(x, freqs_cis, w_qkv, w_proj, mode="coll"):
    nc = _get_nc(mode)
    in_maps = make_in_maps(x, freqs_cis, w_qkv, w_proj, mode)
    res = run_bass_kernel_spmd(nc, in_maps, list(range(NCORES)))
    full = np.empty((B, T, C), np.float32)
    for c in range(NCORES):
        b, h = divmod(c, 2)
        full[b, h * TQ:(h + 1) * TQ, :] = res.results[c]["out"]
    return full
